# revision 1
# baseline (speedup 1.0000x reference)
"""Trainium2 Bass kernel for the GCNEncoder problem.

Strategy:
  - Pure data parallelism: batch 65536 split as 8192 per core across 8 cores.
  - Host-side folding (weights are tiny):
      C1 = kron(A, W1)            -- layer-1 graph-mix + lift fused: (1088, 51)
      C2 = kron(A, W2)            -- layer-2 fused, block-sparse over joint pairs
      D  = Wp1 @ kron(A, W3)      -- layer-3 + pool-layer-1 collapse: (64, 1088)
  - x is transposed host-side to (51, B) so activations stay
    feature-on-partition on device with perfectly contiguous DMA.
  - Device pipeline per 512-sample tile:
      stage A: 9 matmuls (K=51)               -> H1 (128, 9*512) pair layout
      stage B: 18 pair-block matmuls (K<=128) -> H2 (128, 9*512)
      stage C: 9 accumulating matmuls         -> z4 (64, 512)
      stage D: 4 matmuls with z4 as lhsT      -> out (samples, 256) directly
    Bias+ReLU fused into PSUM evacuation, split across ScalarE/VectorE.
    Joints are paired as siblings so stage B needs only 18 blocks (vs 24).
  - float32r matmuls (single-pass fp32, ~1e-4 matmul rel err, 4x faster
    than the fp32 two-pass path).
"""

import os
import sys

for _p in ("/opt/trn_rl_repo", "/root/.axon_site/_ro/trn_rl_repo"):
    if os.path.isdir(_p) and _p not in sys.path:
        sys.path.insert(0, _p)

import numpy as np

from concourse import bacc, mybir, tile
from concourse.bass_utils import run_bass_kernel_spmd

NJ = 17            # joints
DIN = 3            # input dims per joint
H = 64             # hidden per joint
DOUT = 256
NCORES = 8
B_TOTAL = 65536
BC = B_TOTAL // NCORES          # 8192 per core
TILE_N = 512                    # samples per device tile
CHUNKS_PER_TILE = TILE_N // 128  # 4
NTILES = BC // TILE_N           # 16
NCHUNKS = BC // 128             # 64

F32 = mybir.dt.float32
F32R = mybir.dt.float32r
BF16 = mybir.dt.bfloat16
B_BF16 = bool(int(os.environ.get("KERNEL_B_BF16", "0")))
BDT = BF16 if B_BF16 else F32R

# Joint pairing chosen to minimize nonzero pair-blocks of kron(A, W2):
# siblings (nodes sharing a neighbor, never adjacent) share their neighbor
# sets, so the 16 tree edges collapse into 9 unordered pair-pairs (18
# ordered blocks) and no diagonal blocks. Natural pairing gives 24.
PAIRS = [(1, 3), (4, 6), (8, 10), (11, 13), (14, 16), (7, 9), (0, 2), (5, 12), (15,)]
PERM = [j for pq in PAIRS for j in pq]          # joint order, row-block major

LAST_RESULTS = None  # stash of BassKernelResults for test harness introspection


def _build_constants(A, W1, b1, W2, b2, W3, b3, Wp1, bp1, Wp2, bp2):
    """Host-side folding. All fp32 numpy."""
    A = np.asarray(A, np.float32)
    C1 = np.kron(A, np.asarray(W1, np.float32))            # (1088, 51)
    C2 = np.kron(A, np.asarray(W2, np.float32))            # (1088, 1088)
    C3 = np.kron(A, np.asarray(W3, np.float32))            # (1088, 1088)
    D = np.asarray(Wp1, np.float32) @ C3                   # (64, 1088)
    bp1p = (np.asarray(Wp1, np.float32) @ np.tile(np.asarray(b3, np.float32), NJ)
            + np.asarray(bp1, np.float32))                 # (64,)

    # permute joint-major rows into PAIRS order
    perm_rows = np.concatenate([np.arange(j * H, (j + 1) * H) for j in PERM])
    C1 = C1[perm_rows]
    C2 = C2[perm_rows][:, perm_rows]
    D = D[:, perm_rows]

    # G1: lhsT chunks of C1, concatenated along free dim. chunk q is (51, Mq)
    g1 = C1.T.copy()                                       # (51, 1088)

    # G2: nonzero pair blocks of C2, transposed, concatenated along free dim
    row_off = [128 * q for q in range(9)]
    blocks = []   # (q, p, coloff, K, M)
    cols = []
    coloff = 0
    for q, pq in enumerate(PAIRS):
        Mq = H * len(pq)
        for p, pp in enumerate(PAIRS):
            Kp = H * len(pp)
            blk = C2[row_off[q]:row_off[q] + Mq, row_off[p]:row_off[p] + Kp]
            if np.abs(blk).max() == 0.0:
                continue
            t = np.zeros((128, Mq), np.float32)
            t[:Kp, :] = blk.T
            blocks.append((q, p, coloff, Kp, Mq))
            cols.append(t)
            coloff += Mq
    g2 = np.concatenate(cols, axis=1)                      # (128, ~3008)

    # G3: D.T chunks (Kp, 64) at columns 64*p
    g3 = np.zeros((128, 9 * H), np.float32)
    for p, pp in enumerate(PAIRS):
        Kp = H * len(pp)
        g3[:Kp, H * p:H * (p + 1)] = D[:, row_off[p]:row_off[p] + Kp].T

    consts = {
        "g1": g1,
        "g2": g2,
        "g3": g3,
        "wp2t": np.asarray(Wp2, np.float32).T.copy(),      # (64, 256)
        "b1p": np.tile(np.asarray(b1, np.float32), 2).reshape(128, 1).copy(),
        "b2p": np.tile(np.asarray(b2, np.float32), 2).reshape(128, 1).copy(),
        "bp1p": bp1p.reshape(64, 1).copy(),
        "bp2b": np.tile(np.asarray(bp2, np.float32), (128, 2)).copy(),  # (128, 512)
    }
    return consts, blocks


def _build_program(blocks, reps=1):
    probe = os.environ.get("KERNEL_PROBE", "none")
    nc = bacc.Bacc(None)

    x_d = nc.declare_dram_parameter("x", [NJ * DIN, BC], F32R, isOutput=False)
    g1_d = nc.declare_dram_parameter("g1", [NJ * DIN, NJ * H], F32R, isOutput=False)
    g2_cols = max(b[2] + b[4] for b in blocks)
    g2_d = nc.declare_dram_parameter("g2", [128, g2_cols], BDT, isOutput=False)
    g3_d = nc.declare_dram_parameter("g3", [128, 9 * H], F32R, isOutput=False)
    wp2t_d = nc.declare_dram_parameter("wp2t", [H, DOUT], F32R, isOutput=False)
    b1p_d = nc.declare_dram_parameter("b1p", [128, 1], F32, isOutput=False)
    b2p_d = nc.declare_dram_parameter("b2p", [128, 1], F32, isOutput=False)
    bp1p_d = nc.declare_dram_parameter("bp1p", [H, 1], F32, isOutput=False)
    bp2b_d = nc.declare_dram_parameter("bp2b", [128, 512], F32, isOutput=False)
    out_d = nc.declare_dram_parameter("out", [BC, DOUT], F32, isOutput=True)

    out_r = out_d.rearrange("(c p) f -> p c f", p=128)      # (128, 64, 256)

    AF = mybir.ActivationFunctionType
    ALU = mybir.AluOpType

    # blocks grouped by output pair q
    blocks_by_q = [[b for b in blocks if b[0] == q] for q in range(9)]

    with tile.TileContext(nc) as tc:
        with (
            tc.tile_pool(name="const", bufs=1) as cp,
            tc.tile_pool(name="h1", bufs=2) as h1p,
            tc.tile_pool(name="h2", bufs=2) as h2p,
            tc.tile_pool(name="z4", bufs=2) as z4p,
            tc.tile_pool(name="ot", bufs=3) as otp,
            tc.tile_pool(name="psa", bufs=3, space="PSUM") as psa,
            tc.tile_pool(name="psb", bufs=3, space="PSUM") as psb,
            tc.tile_pool(name="psc", bufs=1, space="PSUM") as psc,
            tc.tile_pool(name="psd", bufs=1, space="PSUM") as psd,
        ):
            x_all = cp.tile([NJ * DIN, BC], F32R)
            g1_sb = cp.tile([NJ * DIN, NJ * H], F32R)
            g2_sb = cp.tile([128, g2_cols], BDT)
            g3_sb = cp.tile([128, 9 * H], F32R)
            wp2t_sb = cp.tile([H, DOUT], F32R)
            b1p_sb = cp.tile([128, 1], F32)
            b2p_sb = cp.tile([128, 1], F32)
            bp1p_sb = cp.tile([H, 1], F32)
            bp2b_sb = cp.tile([128, 512], F32)

            nc.sync.dma_start(x_all[:], x_d[:])
            nc.sync.dma_start(g1_sb[:], g1_d[:])
            nc.sync.dma_start(g2_sb[:], g2_d[:])
            nc.sync.dma_start(g3_sb[:], g3_d[:])
            nc.sync.dma_start(wp2t_sb[:], wp2t_d[:])
            nc.sync.dma_start(b1p_sb[:], b1p_d[:])
            nc.sync.dma_start(b2p_sb[:], b2p_d[:])
            nc.sync.dma_start(bp1p_sb[:], bp1p_d[:])
            nc.sync.dma_start(bp2b_sb[:], bp2b_d[:])
            if int(os.environ.get("KERNEL_SALT", "0")):
                salt_sb = cp.tile([128, 1 + int(os.environ["KERNEL_SALT"])], F32)
                nc.gpsimd.memset(salt_sb[:], 0.0)

            def evac_relu_act(dst, src, bias):
                nc.scalar.activation(dst, src, AF.Relu, bias=bias)

            def evac_relu_dve(dst, src, bias):
                nc.vector.tensor_scalar(
                    out=dst, in0=src, scalar1=bias, scalar2=0.0,
                    op0=ALU.add, op1=ALU.max,
                )

            def tile_body(t):
                xt_sb = x_all[:, TILE_N * t:TILE_N * (t + 1)]

                # ---- stage A: H1 = relu(C1 @ x + b1), pair layout
                h1_sb = h1p.tile([128, 9 * TILE_N], BDT)
                for q, pq in enumerate(PAIRS):
                    Mq = H * len(pq)
                    ps_a = psa.tile([128, TILE_N], F32)
                    if probe != "noA":
                        nc.tensor.matmul(
                            ps_a[0:Mq, :],
                            g1_sb[:, 128 * q:128 * q + Mq],
                            xt_sb[:],
                            start=True, stop=True,
                        )
                    dst = h1_sb[0:Mq, TILE_N * q:TILE_N * (q + 1)]
                    if q % 2 == 0:
                        evac_relu_act(dst, ps_a[0:Mq, :], b1p_sb[0:Mq, :])
                    else:
                        evac_relu_dve(dst, ps_a[0:Mq, :], b1p_sb[0:Mq, :])

                # ---- stage B: H2 = relu(C2 @ H1 + b2), pair-block sparse
                h2_sb = h2p.tile([128, 9 * TILE_N], F32R)
                for q, pq in enumerate(PAIRS):
                    Mq = H * len(pq)
                    ps_b = psb.tile([128, TILE_N], F32)
                    bq = blocks_by_q[q]
                    for bi, (_, p, coloff, Kp, Mq2) in enumerate(bq):
                        if probe == "noB":
                            continue
                        if probe == "sameW":
                            coloff = 0
                            Kp = 128
                            Mq_w = 128 if Mq == 128 else Mq
                        nc.tensor.matmul(
                            ps_b[0:Mq, :],
                            g2_sb[0:Kp, coloff:coloff + Mq],
                            h1_sb[0:Kp, TILE_N * p:TILE_N * (p + 1)],
                            start=(bi == 0), stop=(bi == len(bq) - 1),
                        )
                    dst = h2_sb[0:Mq, TILE_N * q:TILE_N * (q + 1)]
                    if q % 2 == 0:
                        evac_relu_act(dst, ps_b[0:Mq, :], b2p_sb[0:Mq, :])
                    else:
                        evac_relu_dve(dst, ps_b[0:Mq, :], b2p_sb[0:Mq, :])

                # ---- stage C: z4 = relu(D @ H2 + bp1'), single accumulator
                ps_c = psc.tile([H, TILE_N], F32)
                for p, pp in enumerate(PAIRS):
                    if probe == "noC":
                        continue
                    Kp = H * len(pp)
                    nc.tensor.matmul(
                        ps_c[:],
                        g3_sb[0:Kp, H * p:H * (p + 1)],
                        h2_sb[0:Kp, TILE_N * p:TILE_N * (p + 1)],
                        start=(p == 0), stop=(p == 8),
                    )
                z4_sb = z4p.tile([H, TILE_N], F32R)
                evac_relu_act(z4_sb[:], ps_c[:], bp1p_sb[:])

                # ---- stage D: out = z4.T @ Wp2T + bp2 (samples on partitions)
                ot_sb = otp.tile([128, CHUNKS_PER_TILE * DOUT], F32)
                for half in range(2):
                    ps_d = psd.tile([128, 512], F32)
                    for k in range(2):
                        kk = 2 * half + k
                        nc.tensor.matmul(
                            ps_d[:, DOUT * k:DOUT * (k + 1)],
                            z4_sb[:, 128 * kk:128 * (kk + 1)],
                            wp2t_sb[:],
                            start=True, stop=True,
                        )
                    nc.vector.tensor_tensor(
                        out=ot_sb[:, 512 * half:512 * (half + 1)],
                        in0=ps_d[:],
                        in1=bp2b_sb[:],
                        op=ALU.add,
                    )
                nc.sync.dma_start(
                    out_r[:, CHUNKS_PER_TILE * t:CHUNKS_PER_TILE * (t + 1), :],
                    ot_sb[:],
                )

            if reps == 1:
                for t in range(NTILES):
                    tile_body(t)
            else:
                with tc.For_i(0, reps, 1):
                    for t in range(NTILES):
                        tile_body(t)

    nc.compile()
    return nc


_CACHE = {}


def kernel(**inputs):
    global LAST_RESULTS
    x = np.ascontiguousarray(np.asarray(inputs["x"], np.float32))
    consts, blocks = _build_constants(
        inputs["A"], inputs["W1"], inputs["b1"], inputs["W2"], inputs["b2"],
        inputs["W3"], inputs["b3"], inputs["Wp1"], inputs["bp1"],
        inputs["Wp2"], inputs["bp2"],
    )

    reps = int(os.environ.get("BENCH_REPS", "1"))
    key = (reps, B_BF16) + tuple(b[:3] for b in blocks)
    if key not in _CACHE:
        _CACHE[key] = _build_program(blocks, reps=reps)
    nc = _CACHE[key]

    if B_BF16:
        import ml_dtypes
        consts["g2"] = consts["g2"].astype(ml_dtypes.bfloat16)
    xf = x.reshape(B_TOTAL, NJ * DIN)
    in_maps = []
    for c in range(NCORES):
        m = dict(consts)
        m["x"] = np.ascontiguousarray(xf[c * BC:(c + 1) * BC].T)
        in_maps.append(m)

    res = run_bass_kernel_spmd(nc, in_maps, list(range(NCORES)))
    LAST_RESULTS = res
    out = np.concatenate([res.results[c]["out"] for c in range(NCORES)], axis=0)
    return out.astype(np.float32, copy=False)



# revision 3
# speedup vs baseline: 1.1158x; 1.1158x over previous
"""Trainium2 Bass kernel for the GCNEncoder problem (v2).

Strategy (vs v1 baseline: all-fp32r, 40 MMs/tile, no array tiling):
  - Pure data parallelism: batch 65536 = 8 cores x 8192; 16 tiles of 512.
  - Host folding:  C1 = kron(A,W1) (1088x51),  C2 = kron(A,W2) block-sparse,
    D = Wp1 @ kron(A,W3) (64x1088).  Joint pairing found by exact search
    minimizes nonzero 128x128 pair-blocks of C2: 16 (provably optimal;
    old pairing had 18).
  - Everything bf16 on the PE (1 col/cycle + FWL weight loads; fp8 was
    measured numerically unsafe: B-only fp8 -> 4.8e-2 rel err vs 2e-2 gate).
  - Stage A (h1 = relu(C1 x + b1)): K=51 -> row-tiled 2x via tile_position:
    two chunks run concurrently on array rows 0-63 / 64-127 (x and g1 are
    staged at partition bases 0 and 64).  9 chunks in 5 slots.
  - Stage B: 16 block MMs (K<=128, M<=128), PSUM-accumulated per output
    chunk; chunk pairs share a 2-bank PSUM tile so evacuation runs as
    [128,1024] ops (amortizes per-op engine overhead).
  - Stage C (z4 = relu(D h2 + bp1')): M=64; optionally col-tiled 2x
    (KC_TILED=1) with a DMA partition-shift merge of the two PSUM halves.
  - Stage D (out = z4^T Wp2^T + bp2): K=64; optionally row-tiled 2x
    (KD_TILED=1, z4 copied to partitions 64-127 by a small SBUF DMA).
  - Bias+ReLU fused into PSUM evacuation, balanced across ScalarE/VectorE.
  - PSUM: one shared 3-slot pool of [128,1024] (6 banks) for A/B/D + a
    1-bank accumulator for C.
"""

import os
import sys

for _p in ("/opt/trn_rl_repo", "/root/.axon_site/_ro/trn_rl_repo"):
    if os.path.isdir(_p) and _p not in sys.path:
        sys.path.insert(0, _p)

import numpy as np
import ml_dtypes

from concourse import bacc, mybir, tile
from concourse.bass_utils import run_bass_kernel_spmd

NJ = 17
DIN = 3
H = 64
DOUT = 256
NCORES = 8
B_TOTAL = 65536
BC = B_TOTAL // NCORES          # 8192 per core
TILE_N = 512
CHUNKS_PER_TILE = TILE_N // 128  # 4
NTILES = BC // TILE_N           # 16

F32 = mybir.dt.float32
BF16 = mybir.dt.bfloat16

C_TILED = bool(int(os.environ.get("KC_TILED", "0")))
D_TILED = bool(int(os.environ.get("KD_TILED", "0")))
D_GPSIMD = bool(int(os.environ.get("KD_GPSIMD", "0")))

# Optimal pairing (exact branch-and-bound): 16 nonzero ordered pair-blocks
# of kron(A, W2).  Singleton joint 10 (leaf).
PAIRS = [(0, 8), (5, 15), (1, 11), (6, 16), (3, 13), (2, 12), (4, 14),
         (7, 9), (10,)]
PERM = [j for pq in PAIRS for j in pq]
NCH = len(PAIRS)                 # 9 chunks
KIN = NJ * DIN                   # 51

LAST_RESULTS = None


def _build_constants(A, W1, b1, W2, b2, W3, b3, Wp1, bp1, Wp2, bp2):
    A = np.asarray(A, np.float32)
    C1 = np.kron(A, np.asarray(W1, np.float32))            # (1088, 51)
    C2 = np.kron(A, np.asarray(W2, np.float32))            # (1088, 1088)
    C3 = np.kron(A, np.asarray(W3, np.float32))
    D = np.asarray(Wp1, np.float32) @ C3                   # (64, 1088)
    bp1p = (np.asarray(Wp1, np.float32) @ np.tile(np.asarray(b3, np.float32), NJ)
            + np.asarray(bp1, np.float32))

    perm_rows = np.concatenate([np.arange(j * H, (j + 1) * H) for j in PERM])
    C1 = C1[perm_rows]
    C2 = C2[perm_rows][:, perm_rows]
    D = D[:, perm_rows]

    Mqs = [H * len(pq) for pq in PAIRS]
    row_off = np.cumsum([0] + Mqs)

    g1 = C1.T.copy()                                       # (51, 1088)

    # stage-B blocks: ordered (q, p) with nonzero coupling
    blocks = []   # (q, p, coloff, Kp, Mq)
    cols = []
    coloff = 0
    for q, pq in enumerate(PAIRS):
        Mq = H * len(pq)
        for p, pp in enumerate(PAIRS):
            Kp = H * len(pp)
            blk = C2[row_off[q]:row_off[q] + Mq, row_off[p]:row_off[p] + Kp]
            if np.abs(blk).max() == 0.0:
                continue
            t = np.zeros((128, Mq), np.float32)
            t[:Kp, :] = blk.T
            blocks.append((q, p, coloff, Kp, Mq))
            cols.append(t)
            coloff += Mq
    g2 = np.concatenate(cols, axis=1)

    # stage-C weights: chunk p occupies g3[:, 64p:64p+64]
    g3 = np.zeros((128, NCH * H), np.float32)
    for p, pp in enumerate(PAIRS):
        Kp = H * len(pp)
        g3[:Kp, H * p:H * (p + 1)] = D[:, row_off[p]:row_off[p] + Kp].T

    def bf(a):
        return np.ascontiguousarray(np.asarray(a, np.float32)).astype(
            ml_dtypes.bfloat16)

    consts = {
        "g1": bf(g1),
        "g2": bf(g2),
        "g3": bf(g3),
        "wp2t": bf(np.asarray(Wp2, np.float32).T),         # (64, 256)
        "b1p": np.tile(np.asarray(b1, np.float32), 2).reshape(128, 1).copy(),
        "b2p": np.tile(np.asarray(b2, np.float32), 2).reshape(128, 1).copy(),
        "bp1p": bp1p.reshape(64, 1).copy(),
        "bp2b": np.tile(np.asarray(bp2, np.float32), (128, 4)).copy(),
    }
    return consts, blocks


def _build_program(blocks, reps=1):
    probe = os.environ.get("KERNEL_PROBE", "none")
    nc = bacc.Bacc(None)

    x_d = nc.declare_dram_parameter("x", [KIN, BC], BF16, isOutput=False)
    g1_d = nc.declare_dram_parameter("g1", [KIN, NJ * H], BF16, isOutput=False)
    g2_cols = max(b[2] + b[4] for b in blocks)
    g2_d = nc.declare_dram_parameter("g2", [128, g2_cols], BF16, isOutput=False)
    g3_d = nc.declare_dram_parameter("g3", [128, NCH * H], BF16, isOutput=False)
    wp2t_d = nc.declare_dram_parameter("wp2t", [H, DOUT], BF16, isOutput=False)
    b1p_d = nc.declare_dram_parameter("b1p", [128, 1], F32, isOutput=False)
    b2p_d = nc.declare_dram_parameter("b2p", [128, 1], F32, isOutput=False)
    bp1p_d = nc.declare_dram_parameter("bp1p", [H, 1], F32, isOutput=False)
    bp2b_d = nc.declare_dram_parameter("bp2b", [128, 4 * DOUT], F32,
                                       isOutput=False)
    out_d = nc.declare_dram_parameter("out", [BC, DOUT], F32, isOutput=True)

    out_r = out_d.rearrange("(c p) f -> p c f", p=128)      # (128, 64, 256)

    AF = mybir.ActivationFunctionType
    ALU = mybir.AluOpType

    blocks_by_q = [[b for b in blocks if b[0] == q] for q in range(NCH)]
    Mqs = [H * len(pq) for pq in PAIRS]
    qoff = np.cumsum([0] + Mqs)        # row offset of chunk q within 1088

    with tile.TileContext(nc) as tc:
        with (
            tc.tile_pool(name="const", bufs=1) as cp,
            tc.tile_pool(name="h1", bufs=2) as h1p,
            tc.tile_pool(name="h2", bufs=2) as h2p,
            tc.tile_pool(name="z4", bufs=2) as z4p,
            tc.tile_pool(name="cm", bufs=2) as cmp_,
            tc.tile_pool(name="ot", bufs=3) as otp,
            tc.tile_pool(name="ps2", bufs=3, space="PSUM") as ps2,   # 6 banks
            tc.tile_pool(name="psc", bufs=1, space="PSUM") as psc,   # 1 bank
        ):
            x_all = cp.tile([128, BC], BF16)
            g1_sb = cp.tile([128, NJ * H], BF16)
            g2_sb = cp.tile([128, g2_cols], BF16)
            g3_sb = cp.tile([128, NCH * H], BF16)
            wp2t_sb = cp.tile([128, DOUT], BF16)
            b1p_sb = cp.tile([128, 1], F32)
            b2p_sb = cp.tile([128, 1], F32)
            bp1p_sb = cp.tile([H, 1], F32)
            bp2b_sb = cp.tile([128, 4 * DOUT], F32)

            # inputs staged at both row-tile bases (partitions 0 and 64)
            nc.sync.dma_start(x_all[0:KIN, :], x_d[:])
            nc.sync.dma_start(x_all[64:64 + KIN, :], x_d[:])
            nc.sync.dma_start(g1_sb[0:KIN, :], g1_d[:])
            nc.sync.dma_start(g1_sb[64:64 + KIN, :], g1_d[:])
            nc.sync.dma_start(g2_sb[:], g2_d[:])
            nc.sync.dma_start(g3_sb[:], g3_d[:])
            nc.sync.dma_start(wp2t_sb[0:H, :], wp2t_d[:])
            if D_TILED:
                nc.sync.dma_start(wp2t_sb[64:128, :], wp2t_d[:])
            nc.sync.dma_start(b1p_sb[:], b1p_d[:])
            nc.sync.dma_start(b2p_sb[:], b2p_d[:])
            nc.sync.dma_start(bp1p_sb[:], bp1p_d[:])
            nc.sync.dma_start(bp2b_sb[:], bp2b_d[:])

            def evac_act(dst, src, bias):
                nc.scalar.activation(dst, src, AF.Relu, bias=bias)

            def evac_dve(dst, src, bias):
                nc.vector.tensor_scalar(
                    out=dst, in0=src, scalar1=bias, scalar2=0.0,
                    op0=ALU.add, op1=ALU.max,
                )

            def tile_body(t):
                xcols = slice(TILE_N * t, TILE_N * (t + 1))

                # ---- stage A: h1 = relu(C1 x + b1); row-tiled 2x, K=51
                h1_sb = h1p.tile([128, NCH * TILE_N], BF16)
                for si in range(5):
                    qa, qb = 2 * si, 2 * si + 1
                    ps_a = ps2.tile([128, 1024], F32, tag="ps2", name="ps_a")
                    if probe != "noA":
                        nc.tensor.matmul(
                            ps_a[0:Mqs[qa], 0:TILE_N],
                            g1_sb[0:KIN, qoff[qa]:qoff[qa] + Mqs[qa]],
                            x_all[0:KIN, xcols],
                            start=True, stop=True,
                        )
                        if qb < NCH:
                            nc.tensor.matmul(
                                ps_a[0:Mqs[qb], TILE_N:2 * TILE_N],
                                g1_sb[64:64 + KIN,
                                      qoff[qb]:qoff[qb] + Mqs[qb]],
                                x_all[64:64 + KIN, xcols],
                                start=True, stop=True,
                            )
                    if qb < NCH:
                        dst = h1_sb[:, TILE_N * qa:TILE_N * (qa + 2)]
                        src = ps_a[:, 0:1024]
                        bias = b1p_sb[:]
                    else:
                        dst = h1_sb[0:Mqs[qa], TILE_N * qa:TILE_N * (qa + 1)]
                        src = ps_a[0:Mqs[qa], 0:TILE_N]
                        bias = b1p_sb[0:Mqs[qa]]
                    (evac_act if si in (0, 2, 4) else evac_dve)(dst, src, bias)

                # ---- stage B: h2 = relu(C2 h1 + b2); 16 block MMs
                h2_sb = h2p.tile([128, NCH * TILE_N], BF16)
                for si in range(5):
                    qa, qb = 2 * si, 2 * si + 1
                    ps_b = ps2.tile([128, 1024], F32, tag="ps2", name="ps_b")
                    for qi, q in enumerate((qa, qb)):
                        if q >= NCH:
                            continue
                        bq = blocks_by_q[q]
                        for bi, (_, p, coloff, Kp, Mq) in enumerate(bq):
                            if probe == "noB":
                                continue
                            nc.tensor.matmul(
                                ps_b[0:Mq, TILE_N * qi:TILE_N * (qi + 1)],
                                g2_sb[0:Kp, coloff:coloff + Mq],
                                h1_sb[0:Kp, TILE_N * p:TILE_N * (p + 1)],
                                start=(bi == 0), stop=(bi == len(bq) - 1),
                            )
                    if qb < NCH:
                        dst = h2_sb[:, TILE_N * qa:TILE_N * (qa + 2)]
                        src = ps_b[:, 0:1024]
                        bias = b2p_sb[:]
                    else:
                        dst = h2_sb[0:Mqs[qa], TILE_N * qa:TILE_N * (qa + 1)]
                        src = ps_b[0:Mqs[qa], 0:TILE_N]
                        bias = b2p_sb[0:Mqs[qa]]
                    (evac_dve if si in (0, 2) else evac_act)(dst, src, bias)

                # ---- stage C: z4 = relu(D h2 + bp1')
                z4_sb = z4p.tile([128, TILE_N], BF16)
                ps_c = psc.tile([128, TILE_N], F32, name="ps_c")
                if not C_TILED:
                    for p in range(NCH):
                        if probe == "noC":
                            continue
                        Kp = Mqs[p]
                        nc.tensor.matmul(
                            ps_c[0:H, :],
                            g3_sb[0:Kp, H * p:H * (p + 1)],
                            h2_sb[0:Kp, TILE_N * p:TILE_N * (p + 1)],
                            start=(p == 0), stop=(p == NCH - 1),
                        )
                    evac_act(z4_sb[0:H, :], ps_c[0:H, :], bp1p_sb[:])
                else:
                    # col-tiled: chunks 0-4 -> psum[0:64], 5-8 -> psum[64:128];
                    # halves merged via a partition-shift SBUF DMA
                    lo, hi = list(range(5)), list(range(5, NCH))
                    for i in range(5):
                        for grp, half in ((lo, 0), (hi, 1)):
                            if i >= len(grp) or probe == "noC":
                                continue
                            p = grp[i]
                            Kp = Mqs[p]
                            nc.tensor.matmul(
                                ps_c[64 * half:64 * half + H, :],
                                g3_sb[0:Kp, H * p:H * (p + 1)],
                                h2_sb[0:Kp, TILE_N * p:TILE_N * (p + 1)],
                                start=(i == 0), stop=(i == len(grp) - 1),
                            )
                    chi = cmp_.tile([128, TILE_N], BF16, tag="chi", name="chi")
                    nc.scalar.copy(chi[64:128, :], ps_c[64:128, :])
                    cmg = cmp_.tile([128, TILE_N], BF16, tag="cmg", name="cmg")
                    nc.sync.dma_start(cmg[0:64, :], chi[64:128, :])
                    zp = cmp_.tile([128, TILE_N], F32, tag="zp", name="zp")
                    nc.vector.tensor_tensor(
                        out=zp[0:64, :], in0=ps_c[0:64, :], in1=cmg[0:64, :],
                        op=ALU.add,
                    )
                    evac_act(z4_sb[0:H, :], zp[0:64, :], bp1p_sb[:])
                if D_TILED:
                    nc.sync.dma_start(z4_sb[64:128, :], z4_sb[0:64, :])

                # ---- stage D: out = z4^T Wp2^T + bp2
                ps_d = ps2.tile([128, 1024], F32, tag="ps2", name="ps_d")
                ot_sb = otp.tile([128, 4 * DOUT], F32)
                if not D_TILED:
                    for c in range(4):
                        nc.tensor.matmul(
                            ps_d[:, DOUT * c:DOUT * (c + 1)],
                            z4_sb[0:H, 128 * c:128 * (c + 1)],
                            wp2t_sb[0:H, :],
                            start=True, stop=True,
                        )
                    eng = nc.gpsimd if D_GPSIMD else nc.vector
                    eng.tensor_tensor(
                        out=ot_sb[:], in0=ps_d[:], in1=bp2b_sb[:], op=ALU.add,
                    )
                else:
                    # row-tiled pairs: (c0@rows0-63 -> bank1, c1@rows64-127 ->
                    # bank2), then (c2 -> bank1, c3 -> bank2)
                    dpos = [0, 512, 256, 768]
                    for sl in range(2):
                        for half in range(2):
                            c = 2 * sl + half
                            nc.tensor.matmul(
                                ps_d[:, dpos[c]:dpos[c] + DOUT],
                                z4_sb[64 * half:64 * half + H,
                                      128 * c:128 * (c + 1)],
                                wp2t_sb[64 * half:64 * half + H, :],
                                start=True, stop=True,
                            )
                    # psd free order is (c0 c2 c1 c3); scatter to ot (c0..c3)
                    ps_v = ps_d[:].rearrange("p (b s f) -> p b s f", b=2,
                                             f=DOUT)
                    ot_v = ot_sb[:].rearrange("p (s b f) -> p b s f", b=2,
                                              f=DOUT)
                    bp_v = bp2b_sb[:].rearrange("p (s b f) -> p b s f", b=2,
                                                f=DOUT)
                    eng = nc.gpsimd if D_GPSIMD else nc.vector
                    eng.tensor_tensor(out=ot_v, in0=ps_v, in1=bp_v,
                                      op=ALU.add)
                nc.sync.dma_start(
                    out_r[:, CHUNKS_PER_TILE * t:CHUNKS_PER_TILE * (t + 1), :],
                    ot_sb[:],
                )

            if reps == 1:
                for t in range(NTILES):
                    tile_body(t)
            else:
                with tc.For_i(0, reps, 1):
                    for t in range(NTILES):
                        tile_body(t)

    nc.compile()
    return nc


_CACHE = {}


def kernel(**inputs):
    global LAST_RESULTS
    x = np.ascontiguousarray(np.asarray(inputs["x"], np.float32))
    consts, blocks = _build_constants(
        inputs["A"], inputs["W1"], inputs["b1"], inputs["W2"], inputs["b2"],
        inputs["W3"], inputs["b3"], inputs["Wp1"], inputs["bp1"],
        inputs["Wp2"], inputs["bp2"],
    )

    reps = int(os.environ.get("BENCH_REPS", "1"))
    key = (reps, C_TILED, D_TILED, D_GPSIMD) + tuple(b[:3] for b in blocks)
    if key not in _CACHE:
        _CACHE[key] = _build_program(blocks, reps=reps)
    nc = _CACHE[key]

    xf = x.reshape(B_TOTAL, KIN).astype(ml_dtypes.bfloat16)
    in_maps = []
    for c in range(NCORES):
        m = dict(consts)
        m["x"] = np.ascontiguousarray(xf[c * BC:(c + 1) * BC].T)
        in_maps.append(m)

    res = run_bass_kernel_spmd(nc, in_maps, list(range(NCORES)))
    LAST_RESULTS = res
    out = np.concatenate([res.results[c]["out"] for c in range(NCORES)], axis=0)
    return out.astype(np.float32, copy=False)


# revision 31
# speedup vs baseline: 1.5296x; 1.3708x over previous
"""Trainium2 Bass kernel for the GCNEncoder problem (v2).

Strategy (vs v1 baseline: all-fp32r, 40 MMs/tile, no array tiling):
  - Pure data parallelism: batch 65536 = 8 cores x 8192; 16 tiles of 512.
  - Host folding:  C1 = kron(A,W1) (1088x51),  C2 = kron(A,W2) block-sparse,
    D = Wp1 @ kron(A,W3) (64x1088).  Joint pairing found by exact search
    minimizes nonzero 128x128 pair-blocks of C2: 16 (provably optimal;
    old pairing had 18).
  - Everything bf16 on the PE (fp8 was measured numerically unsafe:
    B-only fp8 -> 4.8e-2 rel err vs the 2e-2 gate).
  - HW-microbenched per-MM costs on this stack showed K<128 matmuls pay
    ~+90ns and M=64 halves the (unhidden) LDWEIGHTS, so EVERY contraction
    is zero-padded to K=128 host-side (x, g1, wp2t padded; h1/h2/z4
    singleton upper halves zeroed once via preallocated buffers).
  - Stage A: 9 MMs (K=128-padded); stage B: 16 block MMs (provably minimal
    pair-block count), accumulation chains interleaved so consecutive MMs
    never hit the same PSUM region; stage C: 9 chained MMs, M=64 (cheap
    LDWEIGHTS); stage D: 4 MMs alternating PSUM banks.
  - Chunk pairs share a 2-bank PSUM tile so bias+ReLU evacuation runs as
    [128,1024] ops (amortizes the ~170-cycle per-op engine overhead),
    balanced across ScalarE/VectorE.
  - PSUM: one shared 3-slot pool of [128,1024] (6 banks) for A/B/D + a
    1-bank accumulator for C.
  - Measured (32k-rep on-device loop, wall-clock differencing): 184 us
    vs 213 us for the fp32r baseline under the identical protocol;
    rel err 5.9e-3.
"""

import os
import sys

for _p in ("/opt/trn_rl_repo", "/root/.axon_site/_ro/trn_rl_repo"):
    if os.path.isdir(_p) and _p not in sys.path:
        sys.path.insert(0, _p)

import numpy as np
import ml_dtypes

from concourse import bacc, mybir, tile
from concourse.bass_utils import run_bass_kernel_spmd

NJ = 17
DIN = 3
H = 64
DOUT = 256
NCORES = 8
B_TOTAL = 65536
BC = B_TOTAL // NCORES          # 8192 per core
TILE_N = 512
CHUNKS_PER_TILE = TILE_N // 128  # 4
NTILES = BC // TILE_N           # 16

F32 = mybir.dt.float32
BF16 = mybir.dt.bfloat16

def _flags():
    return (
        int(os.environ.get("KC_TILED", "0")),
        bool(int(os.environ.get("KD_TILED", "0"))),
        bool(int(os.environ.get("KD_GPSIMD", "0"))),
        os.environ.get("KERNEL_PROBE", "none"),
        int(os.environ.get("KPS2_BUFS", "3")),
        int(os.environ.get("KREP_UNROLL", "1")),
        bool(int(os.environ.get("KA_FLAT", "0"))),
        bool(int(os.environ.get("KA_PAD", "0"))),
        bool(int(os.environ.get("KB_TILED", "0"))),
    )

# Optimal pairing (exact branch-and-bound): 16 nonzero ordered pair-blocks
# of kron(A, W2).  Singleton joint 10 (leaf).
PAIRS_BLK = [(0, 8), (5, 15), (1, 11), (6, 16), (3, 13), (2, 12), (4, 14),
             (7, 9), (10,)]
# Pairing for the 2x2-tiled stage B (KB_TILED): edge-half categories are
# exactly (8,8,8,8) so the 32 directed 64x64 edge-blocks pack into 8 slots
# of 4 concurrent tile_position MMs.
PAIRS_2X2 = [(12, 6), (1, 2), (8, 0), (3, 9), (15, 7), (13, 11), (14, 4),
             (10, 5), (16,)]
CONNS = [(0, 7), (7, 8), (8, 9), (9, 10), (0, 1), (1, 2), (2, 3), (0, 4),
         (4, 5), (5, 6), (8, 11), (11, 12), (12, 13), (8, 14), (14, 15),
         (15, 16)]
NCH = 9                          # chunks
KIN = NJ * DIN                   # 51


def _pairs():
    return PAIRS_2X2 if _flags()[8] else PAIRS_BLK


def _schedule_2x2():
    """Slot schedule for the 2x2-tiled stage B.

    Returns a list of MM descriptors in emission order:
      (bcol, src_half, src_chunk, dst_half, dst_chunk, start, stop)
    plus the list of directed edges in the same order (for g2 building).
    """
    import collections
    pairs = PAIRS_2X2
    u, chunk = {}, {}
    for ci, pq in enumerate(pairs):
        for hi, j in enumerate(pq):
            u[j] = hi
            chunk[j] = ci
    blocks = []
    for a, b in CONNS:
        blocks.append((a, b))
        blocks.append((b, a))
    cats = [(0, 0), (0, 1), (1, 0), (1, 1)]
    remaining = {c: [] for c in cats}
    for s, d in blocks:
        remaining[(u[s], u[d])].append((s, d))

    def ptile(j):
        return chunk[j] // 2

    remcnt = collections.Counter(ptile(d) for s, d in blocks)
    openset = set()
    order = []
    for _ in range(8):
        for c in cats:
            b = sorted(remaining[c],
                       key=lambda e: (ptile(e[1]) not in openset
                                      and len(openset) >= 2,
                                      ptile(e[1])))[0]
            remaining[c].remove(b)
            order.append(b)
            pt = ptile(b[1])
            openset.add(pt)
            remcnt[pt] -= 1
            if remcnt[pt] == 0:
                openset.discard(pt)
    # start/stop per dest joint
    first, last = {}, {}
    for i, (s, d) in enumerate(order):
        first.setdefault(d, i)
        last[d] = i
    descs = []
    for i, (s, d) in enumerate(order):
        descs.append((64 * i, u[s], chunk[s], u[d], chunk[d],
                      first[d] == i, last[d] == i))
    return descs, order, u, chunk

LAST_RESULTS = None


def _build_constants(A, W1, b1, W2, b2, W3, b3, Wp1, bp1, Wp2, bp2):
    PAIRS = _pairs()
    PERM = [j for pq in PAIRS for j in pq]
    B_2X2 = _flags()[8]
    A = np.asarray(A, np.float32)
    C1 = np.kron(A, np.asarray(W1, np.float32))            # (1088, 51)
    C2 = np.kron(A, np.asarray(W2, np.float32))            # (1088, 1088)
    C3 = np.kron(A, np.asarray(W3, np.float32))
    D = np.asarray(Wp1, np.float32) @ C3                   # (64, 1088)
    bp1p = (np.asarray(Wp1, np.float32) @ np.tile(np.asarray(b3, np.float32), NJ)
            + np.asarray(bp1, np.float32))

    perm_rows = np.concatenate([np.arange(j * H, (j + 1) * H) for j in PERM])
    C1 = C1[perm_rows]
    C2 = C2[perm_rows][:, perm_rows]
    D = D[:, perm_rows]

    Mqs = [H * len(pq) for pq in PAIRS]
    row_off = np.cumsum([0] + Mqs)

    g1 = C1.T.copy()                                       # (51, 1088)

    if B_2X2:
        # 32 directed 64x64 edge-blocks laid out in schedule order;
        # block i at cols [64i:64i+64], rows [64*u(src):+64]
        descs, order, uu, _chunk = _schedule_2x2()
        W2f = np.asarray(W2, np.float32)
        g2 = np.zeros((128, 64 * len(order)), np.float32)
        for i, (s, d) in enumerate(order):
            g2[64 * uu[s]:64 * uu[s] + 64, 64 * i:64 * i + 64] = \
                A[d, s] * W2f.T
        blocks = descs
    else:
        # ordered (q, p) pair-blocks with nonzero coupling
        blocks = []   # (q, p, coloff, Kp, Mq)
        cols = []
        coloff = 0
        for q, pq in enumerate(PAIRS):
            Mq = H * len(pq)
            for p, pp in enumerate(PAIRS):
                Kp = H * len(pp)
                blk = C2[row_off[q]:row_off[q] + Mq,
                         row_off[p]:row_off[p] + Kp]
                if np.abs(blk).max() == 0.0:
                    continue
                t = np.zeros((128, Mq), np.float32)
                t[:Kp, :] = blk.T
                blocks.append((q, p, coloff, Kp, Mq))
                cols.append(t)
                coloff += Mq
        g2 = np.concatenate(cols, axis=1)

    # stage-C weights: chunk p occupies g3[:, 64p:64p+64]
    g3 = np.zeros((128, NCH * H), np.float32)
    for p, pp in enumerate(PAIRS):
        Kp = H * len(pp)
        g3[:Kp, H * p:H * (p + 1)] = D[:, row_off[p]:row_off[p] + Kp].T

    def bf(a):
        return np.ascontiguousarray(np.asarray(a, np.float32)).astype(
            ml_dtypes.bfloat16)

    consts = {
        "g1": bf(g1),
        "g2": bf(g2),
        "g3": bf(g3),
        "wp2t": bf(np.asarray(Wp2, np.float32).T),         # (64, 256)
        "b1p": np.tile(np.asarray(b1, np.float32), 2).reshape(128, 1).copy(),
        "b2p": np.tile(np.asarray(b2, np.float32), 2).reshape(128, 1).copy(),
        "bp1p": bp1p.reshape(64, 1).copy(),
        "bp2b": np.tile(np.asarray(bp2, np.float32), (128, 4)).copy(),
        "id64": np.eye(H, dtype=np.float32).astype(ml_dtypes.bfloat16),
    }
    return consts, blocks


def _build_program(blocks, reps=1):
    (C_TILED, D_TILED, D_GPSIMD, probe, PS2_BUFS, _unroll, A_FLAT,
     A_PAD, B_2X2) = _flags()
    PAIRS = _pairs()
    nc = bacc.Bacc(None)

    x_d = nc.declare_dram_parameter("x", [128, BC], BF16, isOutput=False)
    g1_d = nc.declare_dram_parameter("g1", [128, NJ * H], BF16, isOutput=False)
    g2_cols = (64 * len(blocks) if B_2X2
               else max(b[2] + b[4] for b in blocks))
    g2_d = nc.declare_dram_parameter("g2", [128, g2_cols], BF16, isOutput=False)
    g3_d = nc.declare_dram_parameter("g3", [128, NCH * H], BF16, isOutput=False)
    wp2t_d = nc.declare_dram_parameter("wp2t", [128, DOUT], BF16,
                                       isOutput=False)
    b1p_d = nc.declare_dram_parameter("b1p", [128, 1], F32, isOutput=False)
    b2p_d = nc.declare_dram_parameter("b2p", [128, 1], F32, isOutput=False)
    bp1p_d = nc.declare_dram_parameter("bp1p", [H, 1], F32, isOutput=False)
    bp2b_d = nc.declare_dram_parameter("bp2b", [128, 4 * DOUT], F32,
                                       isOutput=False)
    id64_d = nc.declare_dram_parameter("id64", [H, H], BF16, isOutput=False)
    out_d = nc.declare_dram_parameter("out", [BC, DOUT], F32, isOutput=True)

    out_r = out_d.rearrange("(c p) f -> p c f", p=128)      # (128, 64, 256)

    AF = mybir.ActivationFunctionType
    ALU = mybir.AluOpType

    blocks_by_q = (None if B_2X2 else
                   [[b for b in blocks if b[0] == q] for q in range(NCH)])
    Mqs = [H * len(pq) for pq in PAIRS]
    qoff = np.cumsum([0] + Mqs)        # row offset of chunk q within 1088

    with tile.TileContext(nc) as tc:
        with (
            tc.tile_pool(name="const", bufs=1) as cp,
            tc.tile_pool(name="h1", bufs=2) as h1p,
            tc.tile_pool(name="h2", bufs=2) as h2p,
            tc.tile_pool(name="z4", bufs=2) as z4p,
            tc.tile_pool(name="cm", bufs=2) as cmp_,
            tc.tile_pool(name="ot", bufs=3) as otp,
            tc.tile_pool(name="ps2", bufs=PS2_BUFS, space="PSUM") as ps2,
            tc.tile_pool(name="psc", bufs=1, space="PSUM") as psc,   # 1 bank
        ):
            x_all = cp.tile([128, BC], BF16)
            g1_sb = cp.tile([128, NJ * H], BF16)
            g2_sb = cp.tile([128, g2_cols], BF16)
            g3_sb = cp.tile([128, NCH * H], BF16)
            wp2t_sb = cp.tile([128, DOUT], BF16)
            b1p_sb = cp.tile([128, 1], F32)
            b2p_sb = cp.tile([128, 1], F32)
            bp1p_sb = cp.tile([H, 1], F32)
            bp2b_sb = cp.tile([128, 4 * DOUT], F32)
            id64_sb = cp.tile([128, H], BF16)

            nc.sync.dma_start(x_all[:], x_d[:])
            nc.sync.dma_start(g1_sb[0:128, 0:NJ * H], g1_d[:])
            nc.sync.dma_start(g2_sb[:], g2_d[:])
            nc.sync.dma_start(g3_sb[:], g3_d[:])
            nc.sync.dma_start(wp2t_sb[:], wp2t_d[:])
            nc.sync.dma_start(b1p_sb[:], b1p_d[:])
            nc.sync.dma_start(b2p_sb[:], b2p_d[:])
            nc.sync.dma_start(bp1p_sb[:], bp1p_d[:])
            nc.sync.dma_start(bp2b_sb[:], bp2b_d[:])
            nc.sync.dma_start(id64_sb[64:128, :], id64_d[:])

            z4_bufs = [z4p.tile([128, TILE_N], BF16, tag="z4",
                                name=f"z4_{i}") for i in range(2)]
            for zb in z4_bufs:
                nc.gpsimd.memset(zb[64:128, :], 0.0)
            h1_bufs = [h1p.tile([128, NCH * TILE_N], BF16, tag="h1",
                                name=f"h1_{i}") for i in range(2)]
            h2_bufs = [h2p.tile([128, NCH * TILE_N], BF16, tag="h2",
                                name=f"h2_{i}") for i in range(2)]
            for hb in h1_bufs + h2_bufs:
                nc.gpsimd.memset(hb[64:128, 8 * TILE_N:9 * TILE_N], 0.0)

            def evac_act(dst, src, bias):
                nc.scalar.activation(dst, src, AF.Relu, bias=bias)

            def evac_dve(dst, src, bias):
                nc.vector.tensor_scalar(
                    out=dst, in0=src, scalar1=bias, scalar2=0.0,
                    op0=ALU.add, op1=ALU.max,
                )

            # probe=noX shrinks stage X's matmul N to 128 (keeps dataflow
            # valid; output garbage) to measure that stage's marginal cost
            probes = set(probe.split(","))
            NA = 128 if "noA" in probes else TILE_N
            NB = 128 if "noB" in probes else TILE_N
            NC_ = 128 if "noC" in probes else TILE_N
            ND = 64 if "noD" in probes else DOUT
            NE = 128 if "noEvac" in probes else None

            def tile_body(t):
                x0 = TILE_N * t

                # ---- stage A: h1 = relu(C1 x + b1)
                h1_sb = h1_bufs[t % 2]
                for si in range(5):
                    qa, qb = 2 * si, 2 * si + 1
                    ps_a = ps2.tile([128, 1024], F32, tag="ps2", name="ps_a")
                    nc.tensor.matmul(
                        ps_a[0:Mqs[qa], 0:NA],
                        g1_sb[:, qoff[qa]:qoff[qa] + Mqs[qa]],
                        x_all[:, x0:x0 + NA],
                        start=True, stop=True,
                    )
                    if qb < NCH:
                        nc.tensor.matmul(
                            ps_a[0:Mqs[qb], TILE_N:TILE_N + NA],
                            g1_sb[:, qoff[qb]:qoff[qb] + Mqs[qb]],
                            x_all[:, x0:x0 + NA],
                            start=True, stop=True,
                        )
                    if NE is not None:
                        dst = h1_sb[:, TILE_N * qa:TILE_N * qa + NE]
                        src = ps_a[:, 0:NE]
                        bias = b1p_sb[:]
                    elif qb < NCH:
                        dst = h1_sb[:, TILE_N * qa:TILE_N * (qa + 2)]
                        src = ps_a[:, 0:1024]
                        bias = b1p_sb[:]
                    else:
                        dst = h1_sb[0:Mqs[qa], TILE_N * qa:TILE_N * (qa + 1)]
                        src = ps_a[0:Mqs[qa], 0:TILE_N]
                        bias = b1p_sb[0:Mqs[qa]]
                    (evac_act if si in (0, 2, 4) else evac_dve)(dst, src, bias)

                # ---- stage B: h2 = relu(C2 h1 + b2)
                h2_sb = h2_bufs[t % 2]
                if B_2X2:
                    # 32 directed 64x64 edge-blocks, 4 concurrent per slot
                    # via 2x2 row/col tile_position; pair-tile evacuated as
                    # soon as both of its chunks are complete
                    ptile_of = {}
                    remaining = [0] * 5
                    for (_, _, _, dh, dq, _, st) in blocks:
                        if st:
                            remaining[dq // 2] += 1
                    # chunk 8's pair-tile (pt 4) has only its lower dest
                    nev = 0
                    for (bcol, sh, sp, dh, dq, st, sp_) in blocks:
                        pt = dq // 2
                        if pt not in ptile_of:
                            ptile_of[pt] = ps2.tile(
                                [128, 1024], F32, tag="ps2", name="ps_b")
                        ps_b = ptile_of[pt]
                        qi = dq % 2
                        nc.tensor.matmul(
                            ps_b[64 * dh:64 * dh + 64,
                                 TILE_N * qi:TILE_N * qi + NB],
                            g2_sb[64 * sh:64 * sh + 64, bcol:bcol + 64],
                            h1_sb[64 * sh:64 * sh + 64,
                                  TILE_N * sp:TILE_N * sp + NB],
                            start=st, stop=sp_,
                        )
                        if sp_:
                            remaining[pt] -= 1
                            if remaining[pt] == 0:
                                qa = 2 * pt
                                ps_b = ptile_of.pop(pt)
                                if qa + 1 < NCH:
                                    if NE is not None:
                                        dst = h2_sb[:, TILE_N * qa:
                                                    TILE_N * qa + NE]
                                        sb_ = ps_b[:, 0:NE]
                                    else:
                                        dst = h2_sb[:, TILE_N * qa:
                                                    TILE_N * (qa + 2)]
                                        sb_ = ps_b[:, 0:1024]
                                    bias = b2p_sb[:]
                                else:
                                    w = NE if NE is not None else TILE_N
                                    dst = h2_sb[0:Mqs[qa],
                                                TILE_N * qa:TILE_N * qa + w]
                                    sb_ = ps_b[0:Mqs[qa], 0:w]
                                    bias = b2p_sb[0:Mqs[qa]]
                                (evac_dve if nev % 2 == 0 else evac_act)(
                                    dst, sb_, bias)
                                nev += 1
                elif True:
                    pass
                for si in (() if B_2X2 else range(5)):
                    qa, qb = 2 * si, 2 * si + 1
                    ps_b = ps2.tile([128, 1024], F32, tag="ps2", name="ps_b")
                    # interleave the two accumulation chains so consecutive
                    # MMs never target the same PSUM region (drain overlap)
                    seq = []
                    for qi, q in enumerate((qa, qb)):
                        if q >= NCH:
                            continue
                        bq = blocks_by_q[q]
                        for bi, blk in enumerate(bq):
                            seq.append((qi, bi, len(bq), blk))
                    seq.sort(key=lambda e: (e[1], e[0]))
                    for qi, bi, nb, (_, p, coloff, Kp, Mq) in seq:
                        nc.tensor.matmul(
                            ps_b[0:Mq, TILE_N * qi:TILE_N * qi + NB],
                            g2_sb[0:128, coloff:coloff + Mq],
                            h1_sb[0:128, TILE_N * p:TILE_N * p + NB],
                            start=(bi == 0), stop=(bi == nb - 1),
                        )
                    if NE is not None:
                        dst = h2_sb[:, TILE_N * qa:TILE_N * qa + NE]
                        src = ps_b[:, 0:NE]
                        bias = b2p_sb[:]
                    elif qb < NCH:
                        dst = h2_sb[:, TILE_N * qa:TILE_N * (qa + 2)]
                        src = ps_b[:, 0:1024]
                        bias = b2p_sb[:]
                    else:
                        dst = h2_sb[0:Mqs[qa], TILE_N * qa:TILE_N * (qa + 1)]
                        src = ps_b[0:Mqs[qa], 0:TILE_N]
                        bias = b2p_sb[0:Mqs[qa]]
                    (evac_dve if si in (0, 2) else evac_act)(dst, src, bias)

                # ---- stage C: z4 = relu(D h2 + bp1')
                z4_sb = z4_bufs[t % 2]
                ps_c = psc.tile([128, TILE_N], F32, name="ps_c")
                if not C_TILED:
                    for p in range(NCH):
                        nc.tensor.matmul(
                            ps_c[0:H, 0:NC_],
                            g3_sb[0:128, H * p:H * (p + 1)],
                            h2_sb[0:128, TILE_N * p:TILE_N * p + NC_],
                            start=(p == 0), stop=(p == NCH - 1),
                        )
                    if NE is not None:
                        evac_act(z4_sb[0:H, 0:NE], ps_c[0:H, 0:NE],
                                 bp1p_sb[:])
                    else:
                        evac_act(z4_sb[0:H, :], ps_c[0:H, :], bp1p_sb[:])
                else:
                    # col-tiled: chunks 0-4 -> psum[0:64], 5-8 -> psum[64:128]
                    lo, hi = list(range(5)), list(range(5, NCH))
                    for i in range(5):
                        for grp, half in ((lo, 0), (hi, 1)):
                            if i >= len(grp):
                                continue
                            p = grp[i]
                            Kp = Mqs[p]
                            nc.tensor.matmul(
                                ps_c[64 * half:64 * half + H, 0:NC_],
                                g3_sb[0:Kp, H * p:H * (p + 1)],
                                h2_sb[0:Kp, TILE_N * p:TILE_N * p + NC_],
                                start=(i == 0),
                                stop=(i == len(grp) - 1 and
                                      not (C_TILED == 2 and half == 0)),
                            )
                    chi = cmp_.tile([128, TILE_N], BF16, tag="chi", name="chi")
                    nc.scalar.copy(chi[64:128, 0:NC_], ps_c[64:128, 0:NC_])
                    if C_TILED == 2:
                        # fold hi into the lo accumulation chain via an
                        # identity matmul (PE partition shift); lo chain's
                        # stop comes from this MM
                        nc.tensor.matmul(
                            ps_c[0:H, 0:NC_],
                            id64_sb[64:128, :],
                            chi[64:128, 0:NC_],
                            start=False, stop=True,
                        )
                        evac_act(z4_sb[0:H, 0:NC_], ps_c[0:H, 0:NC_],
                                 bp1p_sb[:])
                    else:
                        # merge via partition-shift SBUF DMA + DVE add
                        cmg = cmp_.tile([128, TILE_N], BF16, tag="cmg",
                                        name="cmg")
                        nc.sync.dma_start(cmg[0:64, 0:NC_],
                                          chi[64:128, 0:NC_])
                        zp = cmp_.tile([128, TILE_N], F32, tag="zp", name="zp")
                        nc.vector.tensor_tensor(
                            out=zp[0:64, 0:NC_], in0=ps_c[0:64, 0:NC_],
                            in1=cmg[0:64, 0:NC_], op=ALU.add,
                        )
                        evac_act(z4_sb[0:H, 0:NC_], zp[0:64, 0:NC_],
                                 bp1p_sb[:])
                if D_TILED:
                    nc.sync.dma_start(z4_sb[64:128, :], z4_sb[0:64, :])

                # ---- stage D: out = z4^T Wp2^T + bp2
                ps_d = ps2.tile([128, 1024], F32, tag="ps2", name="ps_d")
                ot_sb = otp.tile([128, 4 * DOUT], F32)
                if not D_TILED:
                    for c in (0, 2, 1, 3):   # alternate PSUM banks
                        nc.tensor.matmul(
                            ps_d[:, DOUT * c:DOUT * c + ND],
                            z4_sb[:, 128 * c:128 * (c + 1)],
                            wp2t_sb[:, 0:ND],
                            start=True, stop=True,
                        )
                    eng = nc.gpsimd if D_GPSIMD else nc.vector
                    if NE is not None:
                        eng.tensor_tensor(
                            out=ot_sb[:, 0:NE], in0=ps_d[:, 0:NE],
                            in1=bp2b_sb[:, 0:NE], op=ALU.add,
                        )
                    else:
                        eng.tensor_tensor(
                            out=ot_sb[:], in0=ps_d[:], in1=bp2b_sb[:],
                            op=ALU.add,
                        )
                else:
                    # row-tiled pairs: (c0@rows0-63 -> bank1, c1@rows64-127 ->
                    # bank2), then (c2 -> bank1, c3 -> bank2)
                    dpos = [0, 512, 256, 768]
                    for sl in range(2):
                        for half in range(2):
                            c = 2 * sl + half
                            nc.tensor.matmul(
                                ps_d[:, dpos[c]:dpos[c] + DOUT],
                                z4_sb[64 * half:64 * half + H,
                                      128 * c:128 * (c + 1)],
                                wp2t_sb[64 * half:64 * half + H, :],
                                start=True, stop=True,
                            )
                    # psd free order is (c0 c2 c1 c3); scatter to ot (c0..c3)
                    ps_v = ps_d[:].rearrange("p (b s f) -> p b s f", b=2,
                                             f=DOUT)
                    ot_v = ot_sb[:].rearrange("p (s b f) -> p b s f", b=2,
                                              f=DOUT)
                    bp_v = bp2b_sb[:].rearrange("p (s b f) -> p b s f", b=2,
                                                f=DOUT)
                    eng = nc.gpsimd if D_GPSIMD else nc.vector
                    eng.tensor_tensor(out=ot_v, in0=ps_v, in1=bp_v,
                                      op=ALU.add)
                if "noDMA" in probes:
                    nc.sync.dma_start(
                        out_r[:, CHUNKS_PER_TILE * t, :],
                        ot_sb[:, 0:DOUT],
                    )
                else:
                    nc.sync.dma_start(
                        out_r[:, CHUNKS_PER_TILE * t:CHUNKS_PER_TILE * (t + 1), :],
                        ot_sb[:],
                    )

            unroll = int(os.environ.get("KREP_UNROLL", "1"))
            if reps == 1:
                for t in range(NTILES):
                    tile_body(t)
            else:
                with tc.For_i(0, reps, 1):
                    for _ in range(unroll):
                        for t in range(NTILES):
                            tile_body(t)

    nc.compile()
    return nc


_CACHE = {}


def kernel(**inputs):
    global LAST_RESULTS
    x = np.ascontiguousarray(np.asarray(inputs["x"], np.float32))
    consts, blocks = _build_constants(
        inputs["A"], inputs["W1"], inputs["b1"], inputs["W2"], inputs["b2"],
        inputs["W3"], inputs["b3"], inputs["Wp1"], inputs["bp1"],
        inputs["Wp2"], inputs["bp2"],
    )

    reps = int(os.environ.get("BENCH_REPS", "1"))
    key = (reps,) + _flags() + tuple(b[:3] for b in blocks)
    if key not in _CACHE:
        _CACHE[key] = _build_program(blocks, reps=reps)
    nc = _CACHE[key]

    xf = x.reshape(B_TOTAL, KIN).astype(ml_dtypes.bfloat16)
    consts = dict(consts)
    consts["g1"] = np.ascontiguousarray(np.pad(
        consts["g1"], ((0, 128 - KIN), (0, 0))))
    consts["wp2t"] = np.ascontiguousarray(np.pad(
        consts["wp2t"], ((0, 128 - H), (0, 0))))
    xf = np.pad(xf, ((0, 0), (0, 128 - KIN)))
    in_maps = []
    for c in range(NCORES):
        m = dict(consts)
        m["x"] = np.ascontiguousarray(xf[c * BC:(c + 1) * BC].T)
        in_maps.append(m)

    res = run_bass_kernel_spmd(nc, in_maps, list(range(NCORES)))
    LAST_RESULTS = res
    out = np.concatenate([res.results[c]["out"] for c in range(NCORES)], axis=0)
    return out.astype(np.float32, copy=False)
